# revision 41
# baseline (speedup 1.0000x reference)
"""GPTBigCode fused MQA attention block (prefill) on 8 Trainium2 NeuronCores.

Full-problem shapes: x [2,2048,2048], w_attn [2048,2304], w_proj [2048,2048],
H=16 query heads, head_dim=128, 1 shared K/V head (MQA), causal softmax.

Sharding: 2-way data parallel over batch x 4-way tensor parallel over query
heads. Core c handles batch c//4 and query heads 4*(c%4)..4*(c%4)+3; the
single K/V head is replicated. c_proj is row-sharded, so each core emits a
partial [2048,2048] output (bf16); the host gather sums the 4 partials per
batch in f32 (b_proj is added on exactly one core per batch).

Per-core kernel, software-pipelined so the PE array never idles:
  prologue   qkv^T(sc0) = wq^T @ x^T chunk 0 (per-dt-sliced DMAs so matmuls
             start as soon as the first 128-row weight/x slices land)
  loop qi    attention q-chunk qi (scores = k_j @ q, exp on Act engine,
             probs @ v accumulated in PSUM) with projection chunks of
             sc=qi+1 and c_proj chunks of qi-1 interleaved between the
             scores and probs@v matmuls as PE filler while exps run
  All matmul operands are bf16 (1 row/cycle); v is transposed via the DMA
  XBAR instead of the PE; softmax rowsums come from a ones-matmul over a
  bf16 probs accumulator built on the vector engine.
"""

import os
import sys

for _p in ("/opt/trn_rl_repo", "/root/.axon_site/_ro/trn_rl_repo"):
    if os.path.isdir(_p) and _p not in sys.path:
        sys.path.insert(0, _p)
        break

import numpy as np

B, S, D = 2, 2048, 2048
H, HD = 16, 128
P = 128
NH = 4          # query heads per core
DT = D // P     # 16 contraction tiles
CT = NH + 2     # qkv^T c-tiles per core (4 q heads + k + v)
SC = 512        # qkv phase s-chunk (moving free dim)
NSC = S // SC
QC = 512        # attention q chunk
NQC = S // QC
KB = S // P     # 16 k blocks
SCALE = float(1.0 / np.sqrt(np.float32(HD)))

_cache = {}
_last_results = None


def _build():
    import concourse.mybir as mybir
    import concourse.tile as tile
    from concourse import bacc

    F32 = mybir.dt.float32
    BF16 = mybir.dt.bfloat16
    ADD = mybir.AluOpType.add
    MULT = mybir.AluOpType.mult
    EXP = mybir.ActivationFunctionType.Exp

    nc = bacc.Bacc("TRN2", target_bir_lowering=False, debug=False)

    xT = nc.dram_tensor("xT", [D, S], BF16, kind="ExternalInput").ap()
    wq = nc.dram_tensor("wq", [D, CT * P], BF16, kind="ExternalInput").ap()
    bqkv = nc.dram_tensor("bqkv", [CT, P], F32, kind="ExternalInput").ap()
    wp = nc.dram_tensor("wp", [NH * P, D], BF16, kind="ExternalInput").ap()
    bp = nc.dram_tensor("bp", [1, D], F32, kind="ExternalInput").ap()
    out = nc.dram_tensor("out_p", [S, D], BF16, kind="ExternalOutput").ap()

    xT_r = xT.rearrange("(dt p) s -> p dt s", p=P)       # [128, 16, 2048]
    wq_r = wq.rearrange("(dt p) c -> p dt c", p=P)       # [128, 16, 768]
    wp_r = wp.rearrange("(ct p) d -> p ct d", p=P)       # [128, 4, 2048]

    with tile.TileContext(nc) as tc:
        with (
            tc.tile_pool(name="consts", bufs=1) as consts,
            tc.tile_pool(name="xt", bufs=3) as p_xt,
            tc.tile_pool(name="wqp", bufs=1) as p_wq,
            tc.tile_pool(name="qk", bufs=1) as p_qk,
            tc.tile_pool(name="vv", bufs=1) as p_v,
            tc.tile_pool(name="vtmp", bufs=2) as p_vtmp,
            tc.tile_pool(name="probs", bufs=10) as p_probs,
            tc.tile_pool(name="accp", bufs=2) as p_acc,
            tc.tile_pool(name="ot", bufs=2) as p_ot,
            tc.tile_pool(name="outsb", bufs=4) as p_out,
            tc.tile_pool(name="bcastp", bufs=2) as p_bc,
            tc.tile_pool(name="pgen", bufs=2, space="PSUM") as pp_gen,
            tc.tile_pool(name="psc", bufs=2, space="PSUM") as pp_sc,
            tc.tile_pool(name="pout", bufs=4, space="PSUM") as pp_out,
        ):
            # ---- constants ----
            ones = consts.tile([P, 1], BF16)
            nc.vector.memset(ones, 1.0)
            bq_sb = consts.tile([P, CT], F32)
            nc.sync.dma_start(out=bq_sb, in_=bqkv.rearrange("c p -> p c"))
            bp_row = consts.tile([1, D], F32)
            nc.sync.dma_start(out=bp_row, in_=bp)
            bp_sb = consts.tile([P, D], F32)
            nc.gpsimd.partition_broadcast(bp_sb, bp_row[0:1, :])

            # ---- persistent SBUF tensors ----
            wq_t = p_wq.tile([P, DT, CT * P], BF16, name="wq_t")
            wp_sb = consts.tile([P, NH, D], BF16)

            # qkv^T [c, s]: c-tiles 0..3 = q heads, 4 = k^T; v is transposed
            # through the DMA XBAR into [s,128] blocks.
            qkT = p_qk.tile([P, NH + 1, S], BF16, name="qkT")
            v = p_v.tile([P, KB, HD], BF16, name="v")

            xts = [None] * NSC
            DG = 4  # dts per DMA dispatch (dispatch costs ~0.6us of queue)

            def load_xt(sc):
                ssl = slice(sc * SC, (sc + 1) * SC)
                xt = p_xt.tile([P, DT, SC], BF16, name=f"xt_{sc}", tag="xt")
                for dg in range(0, DT, DG):
                    nc.sync.dma_start(
                        out=xt[:, dg:dg + DG, :], in_=xT_r[:, dg:dg + DG, ssl])
                xts[sc] = xt

            def finish_proj_chain(sc, ct, ps):
                ssl = slice(sc * SC, (sc + 1) * SC)
                bias = bq_sb[:, ct:ct + 1].to_broadcast((P, SC))
                if ct < NH + 1:
                    nc.vector.tensor_tensor(
                        out=qkT[:, ct, ssl], in0=ps[:, :SC], in1=bias, op=ADD)
                else:
                    vt = p_vtmp.tile([P, SC], BF16, name=f"vt_{sc}", tag="vt")
                    nc.vector.tensor_tensor(
                        out=vt, in0=ps[:, :SC], in1=bias, op=ADD)
                    for i in range(SC // P):
                        nc.sync.dma_start(
                            out=v[:, sc * (SC // P) + i, :],
                            in_=vt[:, i * P:(i + 1) * P],
                            transpose=True)

            def emit_proj_chain(sc, ct):
                """One qkv^T column-tile for s-chunk sc: 16 accumulating
                matmuls + bias add (+ v DMA-transpose for ct==5)."""
                xt = xts[sc]
                ps = pp_gen.tile([P, 512], F32, tag="gen",
                                 name=f"qkv_ps_{sc}_{ct}")
                for dt_i in range(DT):
                    nc.tensor.matmul(
                        ps[:, :SC],
                        lhsT=wq_t[:, dt_i, ct * P:(ct + 1) * P],
                        rhs=xt[:, dt_i, :],
                        start=(dt_i == 0),
                        stop=(dt_i == DT - 1),
                    )
                finish_proj_chain(sc, ct, ps)

            # attention q-chunks: (q0, qw); the final 512 is split in two
            # 256-halves so half of its c_proj can overlap the other half's
            # attention instead of trailing the kernel
            CHUNKS = [(0, 512), (512, 512), (1024, 512),
                      (1536, 256), (1792, 256)]
            oT = [None] * len(CHUNKS)

            def emit_cproj_chain(ci, dc, st):
                """One c_proj output tile [128 q rows x 512 cols]."""
                q0 = CHUNKS[ci][0]
                dsl = slice(dc * QC, (dc + 1) * QC)
                psp = pp_gen.tile([P, 512], F32, tag="gen",
                                  name=f"pr_ps_{ci}_{dc}_{st}")
                for h in range(NH):
                    nc.tensor.matmul(
                        psp,
                        lhsT=oT[ci][:, h, st * P:(st + 1) * P],
                        rhs=wp_sb[:, h, dsl],
                        start=(h == 0), stop=(h == NH - 1),
                    )
                ob = p_out.tile([P, QC], BF16, name=f"ob_{ci}_{dc}_{st}",
                                tag="ob")
                nc.vector.tensor_tensor(
                    out=ob, in0=psp, in1=bp_sb[:, dsl], op=ADD)
                nc.sync.dma_start(
                    out=out[q0 + st * P: q0 + (st + 1) * P, dsl],
                    in_=ob)

            # ---- prologue: wq split per column-tile (first chains' weights
            # land first), x chunk 0 split per 4-dt group; bulk wp last; the
            # first two proj chains run dt-major so the PE tracks arrivals
            ssl0 = slice(0, SC)
            xt0 = p_xt.tile([P, DT, SC], BF16, name="xt_0", tag="xt")
            xts[0] = xt0
            nc.sync.dma_start(out=wq_t[:, :, 0:P], in_=wq_r[:, :, 0:P])
            nc.sync.dma_start(out=xt0[:, 0:DG, :], in_=xT_r[:, 0:DG, ssl0])
            nc.sync.dma_start(out=wq_t[:, :, P:2 * P],
                              in_=wq_r[:, :, P:2 * P])
            for dg in range(DG, DT, DG):
                nc.sync.dma_start(out=xt0[:, dg:dg + DG, :],
                                  in_=xT_r[:, dg:dg + DG, ssl0])
            for ct in range(2, CT):
                nc.sync.dma_start(out=wq_t[:, :, ct * P:(ct + 1) * P],
                                  in_=wq_r[:, :, ct * P:(ct + 1) * P])
            load_xt(1)
            nc.sync.dma_start(out=wp_sb, in_=wp_r)
            ps01 = [pp_gen.tile([P, 512], F32, tag="gen", name=f"qkv_ps_0_{ct}")
                    for ct in range(2)]
            for dt_i in range(DT):
                for ct in range(2):
                    nc.tensor.matmul(
                        ps01[ct][:, :SC],
                        lhsT=wq_t[:, dt_i, ct * P:(ct + 1) * P],
                        rhs=xt0[:, dt_i, :],
                        start=(dt_i == 0),
                        stop=(dt_i == DT - 1),
                    )
            for ct in range(2):
                finish_proj_chain(0, ct, ps01[ct])
            for ct in range(2, CT):
                emit_proj_chain(0, ct)

            def finalize(ci, acc, po_ap):
                """Softmax denominators + normalization -> oT[ci] (bf16)."""
                q0, qw = CHUNKS[ci]
                oT[ci] = p_ot.tile([P, NH, qw], BF16, name=f"oT_{ci}",
                                   tag="oT")
                for h in range(NH):
                    pss = pp_gen.tile([1, qw], F32, tag="gen",
                                      name=f"sum_ps_{ci}_{h}")
                    nc.tensor.matmul(pss, lhsT=ones, rhs=acc[:, h, :],
                                     start=True, stop=True)
                    rec = p_bc.tile([1, qw], F32, name=f"rec_{ci}_{h}",
                                    tag="rec")
                    nc.vector.reciprocal_approx_fast(out=rec, in_=pss)
                    bc = p_bc.tile([P, qw], F32, name=f"bc_{ci}_{h}",
                                   tag="bc")
                    nc.gpsimd.partition_broadcast(bc, rec[0:1, :])
                    nc.vector.tensor_tensor(
                        out=oT[ci][:, h, :], in0=po_ap(h, slice(0, qw)),
                        in1=bc, op=MULT)

            # ---- main loop over attention q-chunks ----
            for ci, (q0, qw) in enumerate(CHUNKS):
                filler = []
                cproj_led = True
                if ci < NSC - 1:
                    if ci + 2 < NSC:
                        load_xt(ci + 2)
                    filler += [(lambda sc=ci + 1, ct=ct:
                                emit_proj_chain(sc, ct)) for ct in range(CT)]
                    cproj_led = False
                if ci > 0:
                    pqw = CHUNKS[ci - 1][1]
                    filler += [(lambda c=ci - 1, dc=dc, st=st:
                                emit_cproj_chain(c, dc, st))
                               for dc in range(D // QC)
                               for st in range(pqw // P)]

                jmax = (q0 + qw) // P
                nfill = len(filler)
                # spread filler across j-groups; when the queue leads with
                # c_proj chains (which wait on the just-deferred normalize),
                # keep group 0 clear
                quota = [0] * jmax
                for idx in range(nfill):
                    if cproj_led:
                        quota[min(jmax - 1, 1 + (idx * (jmax - 1)) // nfill)] += 1
                    else:
                        quota[(idx * jmax) // nfill] += 1
                fpos = 0

                acc = p_acc.tile([P, NH, qw], BF16, name=f"acc_{ci}",
                                 tag="acc")
                if qw == 256:
                    # pack two heads per PSUM bank: the chunk then holds
                    # only 2 of the 4 pool slots, so its probs@v does not
                    # wait on the previous chunk's normalize
                    po2 = [pp_out.tile([P, 2, 256], F32, tag="po",
                                       name=f"po_{ci}_{hp}")
                           for hp in range(2)]

                    def po_ap(h, sl):
                        return po2[h // 2][:, h % 2, sl]
                else:
                    po = [pp_out.tile([P, qw], F32, tag="po",
                                      name=f"po_{ci}_{h}")
                          for h in range(NH)]

                    def po_ap(h, sl):
                        return po[h][:, sl]
                def pap(e, sl):
                    tile, k = e
                    return tile[:, sl] if k is None else tile[:, k, sl]

                pT0s = None
                for j in range(jmax):
                    t = j - q0 // P
                    off = max(0, t * P)
                    w = qw - off
                    pTs = []
                    if qw == 256:
                        # pair two heads per PSUM bank so one activation
                        # covers both (the scalar engine gates these thin,
                        # filler-poor chunks otherwise)
                        for hp in range(2):
                            ps = pp_sc.tile([P, 2, 256], F32, tag="sc",
                                            name=f"sc_ps_{ci}_{j}_{hp}")
                            for k in range(2):
                                h = 2 * hp + k
                                nc.tensor.matmul(
                                    ps[:, k, :w],
                                    lhsT=qkT[:, NH, j * P:(j + 1) * P],
                                    rhs=qkT[:, h, q0 + off:q0 + qw],
                                    start=True, stop=True,
                                )
                            pT2 = p_probs.tile([P, 2, 256], BF16,
                                               name=f"pT_{ci}_{j}_{hp}",
                                               tag="pT")
                            nc.scalar.activation(pT2[:, :, :w], ps[:, :, :w],
                                                 EXP, scale=SCALE)
                            if t >= 0:
                                for k in range(2):
                                    nc.gpsimd.affine_select(
                                        out=pT2[:, k, 0:P],
                                        in_=pT2[:, k, 0:P],
                                        compare_op=mybir.AluOpType.is_ge,
                                        fill=0.0, base=0,
                                        pattern=[[1, P]],
                                        channel_multiplier=-1,
                                    )
                            pTs += [(pT2, 0), (pT2, 1)]
                    else:
                        for h in range(NH):
                            ps = pp_sc.tile([P, 512], F32, tag="sc",
                                            name=f"sc_ps_{ci}_{j}_{h}")
                            nc.tensor.matmul(
                                ps[:, :w],
                                lhsT=qkT[:, NH, j * P:(j + 1) * P],
                                rhs=qkT[:, h, q0 + off:q0 + qw],
                                start=True, stop=True,
                            )
                            pT = p_probs.tile([P, qw], BF16,
                                              name=f"pT_{ci}_{j}_{h}",
                                              tag="pT")
                            nc.scalar.activation(pT[:, :w], ps[:, :w], EXP,
                                                 scale=SCALE)
                            if t >= 0:
                                # strict causal boundary in the leading block
                                nc.gpsimd.affine_select(
                                    out=pT[:, 0:P], in_=pT[:, 0:P],
                                    compare_op=mybir.AluOpType.is_ge,
                                    fill=0.0, base=0,
                                    pattern=[[1, P]], channel_multiplier=-1,
                                )
                            pTs.append((pT, None))

                    # PE filler while the exps run on the scalar engine
                    for _ in range(quota[j]):
                        filler[fpos]()
                        fpos += 1

                    for h in range(NH):
                        pTw = pap(pTs[h], slice(0, w))
                        # probs accumulator for the softmax denominator;
                        # j==0 is folded into j==1's add
                        if j == 1:
                            if off > 0:
                                nc.vector.tensor_copy(
                                    out=acc[:, h, :off],
                                    in_=pap(pT0s[h], slice(0, off)))
                            nc.vector.tensor_tensor(
                                out=acc[:, h, off:],
                                in0=pap(pT0s[h], slice(off, qw)),
                                in1=pTw, op=ADD)
                        elif j > 1:
                            nc.vector.tensor_tensor(
                                out=acc[:, h, off:], in0=acc[:, h, off:],
                                in1=pTw, op=ADD)
                        nc.tensor.matmul(
                            po_ap(h, slice(off, qw)),
                            lhsT=v[:, j, :],
                            rhs=pTw,
                            start=(j == 0), stop=(j == jmax - 1),
                        )
                    if j == 0:
                        pT0s = pTs
                assert fpos == nfill
                finalize(ci, acc, po_ap)

            # ---- tail: last c_proj half-block ----
            for dc in range(D // QC):
                for st in range(CHUNKS[-1][1] // P):
                    emit_cproj_chain(len(CHUNKS) - 1, dc, st)

    nc.compile()
    return nc


def _get_nc():
    if "nc" not in _cache:
        _cache["nc"] = _build()
    return _cache["nc"]


def kernel(x, w_attn, b_attn, w_proj, b_proj, start_pos=0, **_ignored):
    global _last_results
    import ml_dtypes
    from concourse.bass_utils import run_bass_kernel_spmd

    bf16 = ml_dtypes.bfloat16
    x = np.asarray(x, dtype=np.float32)
    w_attn = np.asarray(w_attn, dtype=np.float32)
    b_attn = np.asarray(b_attn, dtype=np.float32)
    w_proj = np.asarray(w_proj, dtype=np.float32)
    b_proj = np.asarray(b_proj, dtype=np.float32)

    nc = _get_nc()

    in_maps = []
    for c in range(8):
        b, hg = divmod(c, 4)
        qcols = slice(hg * NH * HD, (hg + 1) * NH * HD)
        wq_shard = np.ascontiguousarray(
            np.concatenate([w_attn[:, qcols], w_attn[:, D:D + HD],
                            w_attn[:, D + HD:D + 2 * HD]],
                           axis=1).astype(bf16))
        bq_shard = np.ascontiguousarray(
            np.concatenate([b_attn[qcols], b_attn[D:D + HD],
                            b_attn[D + HD:D + 2 * HD]]).reshape(CT, P))
        in_maps.append({
            "xT": np.ascontiguousarray(x[b].T.astype(bf16)),
            "wq": wq_shard,
            "bqkv": bq_shard,
            "wp": np.ascontiguousarray(
                w_proj[hg * NH * HD:(hg + 1) * NH * HD].astype(bf16)),
            "bp": (b_proj if hg == 0 else np.zeros_like(b_proj)).reshape(1, D),
        })

    res = run_bass_kernel_spmd(nc, in_maps, core_ids=list(range(8)))
    _last_results = res
    parts = [r["out_p"].astype(np.float32) for r in res.results]
    out = np.stack([parts[0] + parts[1] + parts[2] + parts[3],
                    parts[4] + parts[5] + parts[6] + parts[7]])
    return out
